# revision 10
# baseline (speedup 1.0000x reference)
"""Trainium2 Bass kernel for nn_Classifier_87256555586283 (KAN 2-layer MLP). v4

Math: each kan_linear(x) = silu(x) @ base_w.T + einsum('nig,oig->no', B(x), spline_w*scaler)
where B(x) are 8 cubic B-spline bases on a uniform grid (knots 0.4 apart).

Reformulation: with u = clip(2.5*x + 5.5, 0, 11), the 8 bases are an exact linear
combination of 11 one-sided cubes phi_s(u) = relu(u - s)^3, s=0..10.  The 8->11
transform is folded into the weights host-side, so each layer becomes 12
elementwise planes (silu + 11 cubes) feeding a dense contraction with K = 12*768.

Precision: fp16 planes/weights (pre-scaled SP=32 / SW=16) + fp8e4 DoubleRow
cross matmuls carrying both residual terms (Wl8@Ph8 + Wh8@Pl8) for the
large-magnitude planes.  v4 ablation (host-emulated): L1 cross for s=0..6 only,
L2 cross s=0..4; end-to-end rel err 9.6e-3 (budget 2e-2).

Layer 2 (768->2) v4: plane-stationary matmuls.  LDWEIGHTS loads a 128-token
slice of the fp16 plane as the stationary operand; the moving operand is the
tiny [128, 4] weight slice [w2hi(2) | w2lo*256(2)], accumulating PSUM [tok, 4]
per token-piece.  Cross planes add [128, 2] w2hi moving against the
plane-residual stationary.  This replaces the M=2 512-cycle matmuls (280us of
cold PE time) with ~60-cycle N=4 matmuls whose LDWEIGHTS overlap main work.
w2lo is scaled 2^8 into its own psum columns (fp16 subnormal avoidance); the
final combine is one stt: out = hi + 2^-8 * lo.

Engine balance (per-chunk, measured costs: DVE ts 334 / stt 600, ACT 612-700,
GP tt 1230): d32+pr80+pr81 on DVE, e32 (Square) + ph (Copy*SP) on ACT,
p32 products on GpSimd.  One ACT table set (sigmoid_and_others), zero switches.

Sharding: data-parallel over the 16384 tokens across 8 cores, weights
replicated, no collectives.  x^T staged host-side (feature-major).
"""

import math

import numpy as np

# problem constants (hardcoded per contract)
B, S, H, L = 32, 512, 768, 2
NTOK = B * S            # 16384
NCORES = 8
TPC = NTOK // NCORES    # 2048 tokens per core
NTB = 512               # token block (PSUM bank = 512 fp32)
NNT = TPC // NTB        # 4
NPC = NTB // 128        # 4 token pieces per block (L2 stationary)
NS = 11                 # relu-cube feature planes
NIC = H // 128          # 6
NOC = H // 128          # 6
NTYPES = 12             # [base, s0..s10]
NK = NIC * NTYPES       # 72 K-chunks of 128
NCT1 = 6                # L1 cross planes: s = 0..5
NCT2 = 5                # L2 cross planes: s = 0..4
SW = 16.0               # weight scale 2^4
SP = 32.0               # plane scale 2^5
SLO = 256.0             # L2 lo-weight boost 2^8
RT32 = float(np.sqrt(np.float32(32.0)))

_PROGRAM = None


def _basis_transform():
    C = np.zeros((8, 12), np.float64)
    for g in range(8):
        for r in range(5):
            C[g, g + r] = ((-1) ** r) * math.comb(4, r) / 6.0
    return C[:, :11]


def _pack_weights(base_w1, spline_w1, scaler1, base_w2, spline_w2, scaler2):
    import ml_dtypes

    f8t = ml_dtypes.float8_e4m3
    C = _basis_transform()

    def full_w(bw, W, sc):
        Wp = np.einsum(
            "oig,gs->ois",
            W.astype(np.float64) * sc[..., None].astype(np.float64), C,
        ).astype(np.float32)
        return np.concatenate(
            [bw.astype(np.float32)[:, :, None], Wp], axis=2
        )  # (O, H, 12) order [base, s0..s10]

    W1 = full_w(base_w1, spline_w1, scaler1)  # (768, 768, 12)
    W2 = full_w(base_w2, spline_w2, scaler2)  # (2, 768, 12)

    W1s = (W1 * np.float32(SW)).astype(np.float32)
    W1h = W1s.astype(np.float16)
    W1l8 = ((W1s - W1h.astype(np.float32)) * np.float32(SP * SW)).astype(f8t)
    W1h8 = W1s.astype(f8t)

    # main fp16 pack: [128, NK*NOC*128], col ((ic*12+j)*NOC+oc)*128
    w1h = np.empty((128, NK * NOC * 128), np.float16)
    for ic in range(NIC):
        isl = slice(ic * 128, (ic + 1) * 128)
        for j in range(NTYPES):
            k = ic * NTYPES + j
            for oc in range(NOC):
                osl = slice(oc * 128, (oc + 1) * 128)
                w1h[:, (k * NOC + oc) * 128 : (k * NOC + oc + 1) * 128] = (
                    W1h[osl, isl, j].T
                )

    # cross fp8 pack: [128, NIC*2, NCT1*NOC*128]; dim1 = (ic, slot);
    # slot 0 = Wl8 (pairs Ph8), slot 1 = Wh8 (pairs Pl8); j = 1..NCT1 (s=0..5)
    w1c = np.zeros((128, NIC * 2, NCT1 * NOC * 128), f8t)
    for ic in range(NIC):
        isl = slice(ic * 128, (ic + 1) * 128)
        for jc in range(NCT1):
            for oc in range(NOC):
                osl = slice(oc * 128, (oc + 1) * 128)
                csl = slice((jc * NOC + oc) * 128, (jc * NOC + oc + 1) * 128)
                w1c[:, ic * 2 + 0, csl] = W1l8[osl, isl, 1 + jc].T
                w1c[:, ic * 2 + 1, csl] = W1h8[osl, isl, 1 + jc].T

    # L2 moving packs (plane-stationary matmuls):
    # w2m [128, NK, 4] = [w2hi(2) | w2lo*SLO(2)], scale 1/SP so psum = h2.
    W2n = W2.astype(np.float64) / SP          # (2, 768, 12)
    W2hi = W2n.astype(np.float16)
    W2lo = ((W2n - W2hi.astype(np.float64)) * SLO).astype(np.float16)
    w2m = np.zeros((128, NK, 4), np.float16)
    for ic in range(NIC):
        isl = slice(ic * 128, (ic + 1) * 128)
        for j in range(NTYPES):
            k2 = ic * NTYPES + j
            w2m[:, k2, 0:2] = W2hi[:, isl, j].T
            w2m[:, k2, 2:4] = W2lo[:, isl, j].T
    # cross (plane-residual) moving pack: w2c [128, NIC*NCT2, 2] = w2hi of s-plane
    w2c = np.zeros((128, NIC * NCT2, 2), np.float16)
    for ic in range(NIC):
        isl = slice(ic * 128, (ic + 1) * 128)
        for jc in range(NCT2):
            w2c[:, ic * NCT2 + jc, :] = W2hi[:, isl, 1 + jc].T
    return (
        np.ascontiguousarray(w1h),
        np.ascontiguousarray(w1c),
        np.ascontiguousarray(w2m),
        np.ascontiguousarray(w2c),
    )


def _build_program():
    import concourse.bass as bass  # noqa: F401
    import concourse.tile as tile
    from concourse import bacc, mybir

    f32 = mybir.dt.float32
    f16 = mybir.dt.float16
    f8 = mybir.dt.float8e4
    A = mybir.ActivationFunctionType
    OP = mybir.AluOpType
    DR = mybir.MatmulPerfMode.DoubleRow

    nc = bacc.Bacc(None, target_bir_lowering=False, debug=False)
    # activation() float biases need registered const APs ([128,1] SBUF)
    bias_vals = {0.0}
    for s in range(1, NS):
        bias_vals.add(-float(s))
        bias_vals.add(-RT32 * s)
    for val in sorted(bias_vals):
        t = nc.alloc_sbuf_tensor(f"constb-{val}", [128, 1], f32)
        nc.gpsimd.memset(t.ap(), val)
        nc.const_aps.aps[(f32, val)] = t.ap()
    nc.all_engine_barrier()

    xT_d = nc.dram_tensor("xT", [H, TPC], f32, kind="ExternalInput")
    w1h_d = nc.dram_tensor("w1h", [128, NK * NOC * 128], f16, kind="ExternalInput")
    w1c_d = nc.dram_tensor(
        "w1c", [128, NIC * 2, NCT1 * NOC * 128], f8, kind="ExternalInput"
    )
    w2m_d = nc.dram_tensor("w2m", [128, NK, 4], f16, kind="ExternalInput")
    w2c_d = nc.dram_tensor("w2c", [128, NIC * NCT2, 2], f16, kind="ExternalInput")
    out_d = nc.dram_tensor("outT", [TPC, L], f32, kind="ExternalOutput")

    with tile.TileContext(nc) as tc:
        from contextlib import ExitStack

        with ExitStack() as ctx:
            wpool = ctx.enter_context(tc.tile_pool(name="wp", bufs=1))
            w1h_sb = wpool.tile([128, NK * NOC * 128], f16, name="w1h_sb")
            blk = 6 * NOC * 128
            for c in range(12):
                csl = slice(c * blk, (c + 1) * blk)
                nc.sync.dma_start(w1h_sb[:, csl], w1h_d[:, csl])
            w2m_sb = wpool.tile([128, NK, 4], f16, name="w2m_sb")
            w2c_sb = wpool.tile([128, NIC * NCT2, 2], f16, name="w2c_sb")
            nc.sync.dma_start(w2m_sb[:], w2m_d[:])
            nc.sync.dma_start(w2c_sb[:], w2c_d[:])

            psum = ctx.enter_context(tc.tile_pool(name="psum", bufs=1, space="PSUM"))
            l1ps = [
                psum.tile([128, NTB], f32, name=f"l1ps{oc}", tag=f"l1ps{oc}")
                for oc in range(NOC)
            ]
            sm = ctx.enter_context(tc.tile_pool(name="sm", bufs=1))
            wcp = ctx.enter_context(tc.tile_pool(name="wcp", bufs=1))

            def build_chunk(src, ic, ncross, consume_main, consume_cross,
                            small_eng):
                """Produce plane tiles for one (input chunk, token block).
                src: [128, NTB] input (f32 x for L1, f16 y for L2).
                consume_main(j, ph), consume_cross(jc, p32, ph16).
                Cross planes s=0..ncross-1; small planes s=ncross..10."""
                t0 = sm.tile([128, NTB], f32, name="t0", tag="t0", bufs=3)
                u = sm.tile([128, NTB], f32, name="u", tag="u", bufs=3)
                sg = sm.tile([128, NTB], f32, name="sg", tag="sg", bufs=3)
                nc.vector.tensor_scalar(t0[:], src[:], 2.5, 5.5, OP.mult, OP.add)
                nc.vector.tensor_scalar(u[:], t0[:], 0.0, 11.0, OP.max, OP.min)
                nc.scalar.activation(sg[:], src[:], A.Sigmoid)
                pb = sm.tile([128, NTB], f16, name="pb", tag="p16", bufs=5)
                nc.vector.scalar_tensor_tensor(
                    pb[:], src[:], SP, sg[:], OP.mult, OP.mult
                )
                consume_main(0, pb)

                for s in range(ncross):  # cross planes
                    if s == 0:
                        d32 = u
                    else:
                        d32 = sm.tile([128, NTB], f32, name=f"d{s}", tag="d32",
                                      bufs=3)
                        nc.vector.tensor_scalar(
                            d32[:], u[:], -float(s), 0.0, OP.add, OP.max
                        )
                    e32 = sm.tile([128, NTB], f32, name=f"e{s}", tag="e32", bufs=3)
                    nc.scalar.activation(
                        e32[:], u[:], A.Square, bias=-float(s), scale=1.0
                    )
                    p32 = sm.tile([128, NTB], f32, name=f"p{s}", tag="p32", bufs=3)
                    if s % 3 == 2:
                        nc.vector.tensor_tensor(p32[:], e32[:], d32[:], OP.mult)
                    else:
                        nc.gpsimd.tensor_tensor(p32[:], e32[:], d32[:], OP.mult)
                    ph = sm.tile([128, NTB], f16, name=f"ph{s}", tag="p16", bufs=5)
                    nc.scalar.mul(ph[:], p32[:], SP)
                    consume_main(1 + s, ph)
                    consume_cross(s, p32, ph)

                for s in range(ncross, NS):  # small planes: fp16-only
                    d32 = sm.tile([128, NTB], f32, name=f"d{s}", tag="d32", bufs=3)
                    e32 = sm.tile([128, NTB], f32, name=f"e{s}", tag="e32", bufs=3)
                    nc.vector.tensor_scalar(
                        d32[:], u[:], -float(s), 0.0, OP.add, OP.max
                    )
                    nc.scalar.activation(
                        e32[:], u[:], A.Square, bias=-RT32 * s, scale=RT32
                    )
                    ph = sm.tile([128, NTB], f16, name=f"ph{s}", tag="p16", bufs=5)
                    if small_eng == "g":
                        nc.gpsimd.tensor_tensor(ph[:], e32[:], d32[:], OP.mult)
                    else:
                        nc.vector.tensor_tensor(ph[:], e32[:], d32[:], OP.mult)
                    consume_main(1 + s, ph)

            for nt in range(NNT):
                tsl = slice(nt * NTB, (nt + 1) * NTB)
                y16s = []

                l2ps = psum.tile([128, 16], f32, name="l2ps", tag="l2ps", bufs=2)

                # ---------------- Layer 1 ----------------
                for ic in range(NIC):
                    wc = wcp.tile([128, 2, NCT1 * NOC * 128], f8, name="wc",
                                  tag="wc", bufs=2)
                    nc.sync.dma_start(
                        wc[:, :, :], w1c_d[:, ic * 2 : ic * 2 + 2, :]
                    )
                    xt = sm.tile([128, NTB], f32, name="xt", tag="xt", bufs=3)
                    nc.sync.dma_start(xt[:], xT_d[ic * 128 : (ic + 1) * 128, tsl])

                    def l1_main(j, ph, ic=ic):
                        k = ic * NTYPES + j
                        for oc in range(NOC):
                            wof = slice((k * NOC + oc) * 128,
                                        (k * NOC + oc + 1) * 128)
                            nc.tensor.matmul(
                                l1ps[oc][:], w1h_sb[:, wof], ph[:],
                                start=(k == 0), stop=(k == NK - 1),
                            )

                    def l1_cross(jc, p32, ph16, ic=ic, wc=wc):
                        pr8 = sm.tile([128, 2, NTB], f8, name="pr8", tag="pr8",
                                      bufs=4)
                        nc.vector.tensor_scalar(
                            pr8[:, 0, :], ph16[:], 1.0 / (SW * SP), None, OP.mult
                        )
                        nc.vector.scalar_tensor_tensor(
                            pr8[:, 1, :], p32[:], SP, ph16[:],
                            OP.mult, OP.subtract,
                        )
                        for oc in range(NOC):
                            wof = slice((jc * NOC + oc) * 128,
                                        (jc * NOC + oc + 1) * 128)
                            nc.tensor.matmul(
                                l1ps[oc][:], wc[:, :, wof], pr8[:, :, :],
                                start=False, stop=False, perf_mode=DR,
                            )

                    build_chunk(xt, ic, NCT1, l1_main, l1_cross, small_eng="g")

                # drain: gelu(h) = h*(0.5+0.5*erf(h/sqrt2)), h = psum/512
                for oc in range(NOC):
                    er = sm.tile([128, NTB], f32, name="er", tag="er", bufs=2)
                    nc.scalar.activation(
                        er[:], l1ps[oc][:], A.Erf,
                        scale=0.7071067811865476 / (SW * SP),
                    )
                    tg = sm.tile([128, NTB], f32, name="tg", tag="tg", bufs=2)
                    nc.vector.tensor_scalar(
                        tg[:], er[:], 0.5 / (SW * SP), 0.5 / (SW * SP),
                        OP.mult, OP.add,
                    )
                    y16 = sm.tile([128, NTB], f16, name=f"y{oc}", tag=f"y{oc}",
                                  bufs=2)
                    nc.vector.tensor_tensor(y16[:], l1ps[oc][:], tg[:], OP.mult)
                    y16s.append(y16)

                # ---------------- Layer 2 (plane-stationary) ----------------
                for ic in range(NIC):
                    def l2_main(j, ph, ic=ic):
                        k2 = ic * NTYPES + j
                        for p in range(NPC):
                            # start=True clears has_written BANK-wide, so only
                            # the very first matmul of the bank may carry it;
                            # later pieces' first writes land on hw=0 and
                            # overwrite (= write) correctly.
                            nc.tensor.matmul(
                                l2ps[0:128, p * 4 : p * 4 + 4],
                                ph[:, p * 128 : (p + 1) * 128],
                                w2m_sb[:, k2, :],
                                start=(k2 == 0 and p == 0),
                                stop=(k2 == NK - 1),
                            )

                    def l2_cross(jc, p32, ph16, ic=ic):
                        pl = sm.tile([128, NTB], f16, name="pl", tag="pl16",
                                     bufs=3)
                        nc.vector.scalar_tensor_tensor(
                            pl[:], p32[:], SP, ph16[:], OP.mult, OP.subtract
                        )
                        for p in range(NPC):
                            nc.tensor.matmul(
                                l2ps[0:128, p * 4 : p * 4 + 2],
                                pl[:, p * 128 : (p + 1) * 128],
                                w2c_sb[:, ic * NCT2 + jc, :],
                                start=False,
                                stop=False,
                            )

                    build_chunk(y16s[ic], ic, NCT2, l2_main, l2_cross,
                                small_eng="g")

                # combine hi + lo/SLO cols -> [128, NPC*2] and store
                obl = sm.tile([128, NPC, L], f32, name="obl", tag="obl", bufs=2)
                nc.vector.tensor_scalar(
                    obl[:, :, :],
                    l2ps[0:128, :].rearrange("p (n f) -> p n f", f=4)[:, :, 2:4],
                    1.0 / SLO, None, OP.mult,
                )
                ob = sm.tile([128, NPC, L], f32, name="ob", tag="ob", bufs=2)
                nc.vector.tensor_tensor(
                    ob[:, :, :], obl[:, :, :],
                    l2ps[0:128, :].rearrange("p (n f) -> p n f", f=4)[:, :, 0:2],
                    OP.add,
                )
                nc.sync.dma_start(
                    out_d[nt * NTB : (nt + 1) * NTB, :].rearrange(
                        "(n q) f -> q n f", q=128
                    ),
                    ob[:, :, :],
                )

    nc.compile()
    return nc


def _get_program():
    global _PROGRAM
    if _PROGRAM is None:
        _PROGRAM = _build_program()
    return _PROGRAM


def run(hidden, base_w1, spline_w1, scaler1, base_w2, spline_w2, scaler2, **kw):
    """Builds inputs, runs the SPMD kernel on 8 cores. Returns (output, results)."""
    from concourse.bass_utils import run_bass_kernel_spmd

    nc = _get_program()
    x = np.ascontiguousarray(np.asarray(hidden, dtype=np.float32).reshape(NTOK, H))
    w1h, w1c, w2m, w2c = _pack_weights(
        np.asarray(base_w1), np.asarray(spline_w1), np.asarray(scaler1),
        np.asarray(base_w2), np.asarray(spline_w2), np.asarray(scaler2),
    )
    in_maps = []
    for c in range(NCORES):
        xT = np.ascontiguousarray(x[c * TPC : (c + 1) * TPC].T)
        in_maps.append(
            {"xT": xT, "w1h": w1h, "w1c": w1c, "w2m": w2m, "w2c": w2c}
        )
    res = run_bass_kernel_spmd(nc, in_maps, list(range(NCORES)), **kw)
    outs = [r["outT"] for r in res.results]  # each (2048, 2)
    out = np.concatenate(outs, axis=0).reshape(B, S, L).astype(np.float32)
    return out, res


def kernel(**inputs):
    out, _ = run(**inputs)
    return out


# revision 11
# speedup vs baseline: 1.0361x; 1.0361x over previous
"""Trainium2 Bass kernel for nn_Classifier_87256555586283 (KAN 2-layer MLP). v4

Math: each kan_linear(x) = silu(x) @ base_w.T + einsum('nig,oig->no', B(x), spline_w*scaler)
where B(x) are 8 cubic B-spline bases on a uniform grid (knots 0.4 apart).

Reformulation: with u = clip(2.5*x + 5.5, 0, 11), the 8 bases are an exact linear
combination of 11 one-sided cubes phi_s(u) = relu(u - s)^3, s=0..10.  The 8->11
transform is folded into the weights host-side, so each layer becomes 12
elementwise planes (silu + 11 cubes) feeding a dense contraction with K = 12*768.

Precision: fp16 planes/weights (pre-scaled SP=32 / SW=16) + fp8e4 DoubleRow
cross matmuls carrying both residual terms (Wl8@Ph8 + Wh8@Pl8) for the
large-magnitude planes.  v4 ablation (host-emulated): L1 cross for s=0..6 only,
L2 cross s=0..4; end-to-end rel err 9.6e-3 (budget 2e-2).

Layer 2 (768->2) v4: plane-stationary matmuls.  LDWEIGHTS loads a 128-token
slice of the fp16 plane as the stationary operand; the moving operand is the
tiny [128, 4] weight slice [w2hi(2) | w2lo*256(2)], accumulating PSUM [tok, 4]
per token-piece.  Cross planes add [128, 2] w2hi moving against the
plane-residual stationary.  This replaces the M=2 512-cycle matmuls (280us of
cold PE time) with ~60-cycle N=4 matmuls whose LDWEIGHTS overlap main work.
w2lo is scaled 2^8 into its own psum columns (fp16 subnormal avoidance); the
final combine is one stt: out = hi + 2^-8 * lo.

Engine balance (per-chunk, measured costs: DVE ts 334 / stt 600, ACT 612-700,
GP tt 1230): d32+pr80+pr81 on DVE, e32 (Square) + ph (Copy*SP) on ACT,
p32 products on GpSimd.  One ACT table set (sigmoid_and_others), zero switches.

Sharding: data-parallel over the 16384 tokens across 8 cores, weights
replicated, no collectives.  x^T staged host-side (feature-major).
"""

import math

import numpy as np

# problem constants (hardcoded per contract)
B, S, H, L = 32, 512, 768, 2
NTOK = B * S            # 16384
NCORES = 8
TPC = NTOK // NCORES    # 2048 tokens per core
NTB = 512               # token block (PSUM bank = 512 fp32)
NNT = TPC // NTB        # 4
NPC = NTB // 128        # 4 token pieces per block (L2 stationary)
NS = 11                 # relu-cube feature planes
NIC = H // 128          # 6
NOC = H // 128          # 6
NTYPES = 12             # [base, s0..s10]
NK = NIC * NTYPES       # 72 K-chunks of 128
NCT1 = 6                # L1 cross planes: s = 0..5
NCT2 = 5                # L2 cross planes: s = 0..4
SW = 16.0               # weight scale 2^4
SP = 32.0               # plane scale 2^5
SLO = 256.0             # L2 lo-weight boost 2^8
RT32 = float(np.sqrt(np.float32(32.0)))

_PROGRAM = None


def _basis_transform():
    C = np.zeros((8, 12), np.float64)
    for g in range(8):
        for r in range(5):
            C[g, g + r] = ((-1) ** r) * math.comb(4, r) / 6.0
    return C[:, :11]


def _pack_weights(base_w1, spline_w1, scaler1, base_w2, spline_w2, scaler2):
    import ml_dtypes

    f8t = ml_dtypes.float8_e4m3
    C = _basis_transform()

    def full_w(bw, W, sc):
        Wp = np.einsum(
            "oig,gs->ois",
            W.astype(np.float64) * sc[..., None].astype(np.float64), C,
        ).astype(np.float32)
        return np.concatenate(
            [bw.astype(np.float32)[:, :, None], Wp], axis=2
        )  # (O, H, 12) order [base, s0..s10]

    W1 = full_w(base_w1, spline_w1, scaler1)  # (768, 768, 12)
    W2 = full_w(base_w2, spline_w2, scaler2)  # (2, 768, 12)

    W1s = (W1 * np.float32(SW)).astype(np.float32)
    W1h = W1s.astype(np.float16)
    W1l8 = ((W1s - W1h.astype(np.float32)) * np.float32(SP * SW)).astype(f8t)
    W1h8 = W1s.astype(f8t)

    # main fp16 pack: [128, NK*NOC*128], col ((ic*12+j)*NOC+oc)*128
    w1h = np.empty((128, NK * NOC * 128), np.float16)
    for ic in range(NIC):
        isl = slice(ic * 128, (ic + 1) * 128)
        for j in range(NTYPES):
            k = ic * NTYPES + j
            for oc in range(NOC):
                osl = slice(oc * 128, (oc + 1) * 128)
                w1h[:, (k * NOC + oc) * 128 : (k * NOC + oc + 1) * 128] = (
                    W1h[osl, isl, j].T
                )

    # cross fp8 pack: [128, NIC*2, NCT1*NOC*128]; dim1 = (ic, slot);
    # slot 0 = Wl8 (pairs Ph8), slot 1 = Wh8 (pairs Pl8); j = 1..NCT1 (s=0..5)
    w1c = np.zeros((128, NIC * 2, NCT1 * NOC * 128), f8t)
    for ic in range(NIC):
        isl = slice(ic * 128, (ic + 1) * 128)
        for jc in range(NCT1):
            for oc in range(NOC):
                osl = slice(oc * 128, (oc + 1) * 128)
                csl = slice((jc * NOC + oc) * 128, (jc * NOC + oc + 1) * 128)
                w1c[:, ic * 2 + 0, csl] = W1l8[osl, isl, 1 + jc].T
                w1c[:, ic * 2 + 1, csl] = W1h8[osl, isl, 1 + jc].T

    # L2 moving packs (plane-stationary matmuls):
    # w2m [128, NK, 4] = [w2hi(2) | w2lo*SLO(2)], scale 1/SP so psum = h2.
    W2n = W2.astype(np.float64) / SP          # (2, 768, 12)
    W2hi = W2n.astype(np.float16)
    W2lo = ((W2n - W2hi.astype(np.float64)) * SLO).astype(np.float16)
    w2m = np.zeros((128, NK, 4), np.float16)
    for ic in range(NIC):
        isl = slice(ic * 128, (ic + 1) * 128)
        for j in range(NTYPES):
            k2 = ic * NTYPES + j
            w2m[:, k2, 0:2] = W2hi[:, isl, j].T
            w2m[:, k2, 2:4] = W2lo[:, isl, j].T
    # cross (plane-residual) moving pack: w2c [128, NIC*NCT2, 2] = w2hi of s-plane
    w2c = np.zeros((128, NIC * NCT2, 2), np.float16)
    for ic in range(NIC):
        isl = slice(ic * 128, (ic + 1) * 128)
        for jc in range(NCT2):
            w2c[:, ic * NCT2 + jc, :] = W2hi[:, isl, 1 + jc].T
    return (
        np.ascontiguousarray(w1h),
        np.ascontiguousarray(w1c),
        np.ascontiguousarray(w2m),
        np.ascontiguousarray(w2c),
    )


def _build_program():
    import concourse.bass as bass  # noqa: F401
    import concourse.tile as tile
    from concourse import bacc, mybir

    f32 = mybir.dt.float32
    f16 = mybir.dt.float16
    f8 = mybir.dt.float8e4
    A = mybir.ActivationFunctionType
    OP = mybir.AluOpType
    DR = mybir.MatmulPerfMode.DoubleRow

    nc = bacc.Bacc(None, target_bir_lowering=False, debug=False)
    # activation() float biases need registered const APs ([128,1] SBUF)
    bias_vals = {0.0}
    for s in range(1, NS):
        bias_vals.add(-float(s))
        bias_vals.add(-RT32 * s)
    for val in sorted(bias_vals):
        t = nc.alloc_sbuf_tensor(f"constb-{val}", [128, 1], f32)
        nc.gpsimd.memset(t.ap(), val)
        nc.const_aps.aps[(f32, val)] = t.ap()
    nc.all_engine_barrier()

    xT_d = nc.dram_tensor("xT", [H, TPC], f32, kind="ExternalInput")
    w1h_d = nc.dram_tensor("w1h", [128, NK * NOC * 128], f16, kind="ExternalInput")
    w1c_d = nc.dram_tensor(
        "w1c", [128, NIC * 2, NCT1 * NOC * 128], f8, kind="ExternalInput"
    )
    w2m_d = nc.dram_tensor("w2m", [128, NK, 4], f16, kind="ExternalInput")
    w2c_d = nc.dram_tensor("w2c", [128, NIC * NCT2, 2], f16, kind="ExternalInput")
    out_d = nc.dram_tensor("outT", [TPC, L], f32, kind="ExternalOutput")

    with tile.TileContext(nc) as tc:
        from contextlib import ExitStack

        with ExitStack() as ctx:
            wpool = ctx.enter_context(tc.tile_pool(name="wp", bufs=1))
            w1h_sb = wpool.tile([128, NK * NOC * 128], f16, name="w1h_sb")
            blk = 6 * NOC * 128
            for c in range(12):
                csl = slice(c * blk, (c + 1) * blk)
                nc.sync.dma_start(w1h_sb[:, csl], w1h_d[:, csl])
            w2m_sb = wpool.tile([128, NK, 4], f16, name="w2m_sb")
            w2c_sb = wpool.tile([128, NIC * NCT2, 2], f16, name="w2c_sb")
            nc.sync.dma_start(w2m_sb[:], w2m_d[:])
            nc.sync.dma_start(w2c_sb[:], w2c_d[:])

            psum = ctx.enter_context(tc.tile_pool(name="psum", bufs=1, space="PSUM"))
            l1ps = [
                psum.tile([128, NTB], f32, name=f"l1ps{oc}", tag=f"l1ps{oc}")
                for oc in range(NOC)
            ]
            sm = ctx.enter_context(tc.tile_pool(name="sm", bufs=1))
            wcp = ctx.enter_context(tc.tile_pool(name="wcp", bufs=1))

            def build_chunk(src, ic, ncross, consume_main, consume_cross,
                            small_eng):
                """Produce plane tiles for one (input chunk, token block).
                src: [128, NTB] input (f32 x for L1, f16 y for L2).
                consume_main(j, ph), consume_cross(jc, p32, ph16).
                Cross planes s=0..ncross-1; small planes s=ncross..10."""
                t0 = sm.tile([128, NTB], f32, name="t0", tag="t0", bufs=3)
                u = sm.tile([128, NTB], f32, name="u", tag="u", bufs=3)
                sg = sm.tile([128, NTB], f32, name="sg", tag="sg", bufs=3)
                nc.vector.tensor_scalar(t0[:], src[:], 2.5, 5.5, OP.mult, OP.add)
                nc.vector.tensor_scalar(u[:], t0[:], 0.0, 11.0, OP.max, OP.min)
                nc.scalar.activation(sg[:], src[:], A.Sigmoid)
                pb = sm.tile([128, NTB], f16, name="pb", tag="p16", bufs=5)
                nc.vector.scalar_tensor_tensor(
                    pb[:], src[:], SP, sg[:], OP.mult, OP.mult
                )
                consume_main(0, pb)

                for s in range(ncross):  # cross planes
                    if s == 0:
                        d32 = u
                    else:
                        d32 = sm.tile([128, NTB], f32, name=f"d{s}", tag="d32",
                                      bufs=3)
                        if s % 2 == 1:
                            nc.scalar.activation(
                                d32[:], u[:], A.Relu, bias=-float(s), scale=1.0
                            )
                        else:
                            nc.vector.tensor_scalar(
                                d32[:], u[:], -float(s), 0.0, OP.add, OP.max
                            )
                    e32 = sm.tile([128, NTB], f32, name=f"e{s}", tag="e32", bufs=3)
                    nc.scalar.activation(
                        e32[:], u[:], A.Square, bias=-float(s), scale=1.0
                    )
                    p32 = sm.tile([128, NTB], f32, name=f"p{s}", tag="p32", bufs=3)
                    if s % 3 == 2:
                        nc.vector.tensor_tensor(p32[:], e32[:], d32[:], OP.mult)
                    else:
                        nc.gpsimd.tensor_tensor(p32[:], e32[:], d32[:], OP.mult)
                    ph = sm.tile([128, NTB], f16, name=f"ph{s}", tag="p16", bufs=5)
                    nc.scalar.mul(ph[:], p32[:], SP)
                    consume_main(1 + s, ph)
                    consume_cross(s, p32, ph)

                for s in range(ncross, NS):  # small planes: fp16-only
                    d32 = sm.tile([128, NTB], f32, name=f"d{s}", tag="d32", bufs=3)
                    e32 = sm.tile([128, NTB], f32, name=f"e{s}", tag="e32", bufs=3)
                    if s % 2 == 1:
                        nc.scalar.activation(
                            d32[:], u[:], A.Relu, bias=-float(s), scale=1.0
                        )
                    else:
                        nc.vector.tensor_scalar(
                            d32[:], u[:], -float(s), 0.0, OP.add, OP.max
                        )
                    nc.scalar.activation(
                        e32[:], u[:], A.Square, bias=-RT32 * s, scale=RT32
                    )
                    ph = sm.tile([128, NTB], f16, name=f"ph{s}", tag="p16", bufs=5)
                    if small_eng == "g":
                        nc.gpsimd.tensor_tensor(ph[:], e32[:], d32[:], OP.mult)
                    else:
                        nc.vector.tensor_tensor(ph[:], e32[:], d32[:], OP.mult)
                    consume_main(1 + s, ph)

            for nt in range(NNT):
                tsl = slice(nt * NTB, (nt + 1) * NTB)
                y16s = []

                l2ps = psum.tile([128, 16], f32, name="l2ps", tag="l2ps", bufs=2)

                # ---------------- Layer 1 ----------------
                for ic in range(NIC):
                    wc = wcp.tile([128, 2, NCT1 * NOC * 128], f8, name="wc",
                                  tag="wc", bufs=2)
                    nc.sync.dma_start(
                        wc[:, :, :], w1c_d[:, ic * 2 : ic * 2 + 2, :]
                    )
                    xt = sm.tile([128, NTB], f32, name="xt", tag="xt", bufs=3)
                    nc.sync.dma_start(xt[:], xT_d[ic * 128 : (ic + 1) * 128, tsl])

                    def l1_main(j, ph, ic=ic):
                        k = ic * NTYPES + j
                        for oc in range(NOC):
                            wof = slice((k * NOC + oc) * 128,
                                        (k * NOC + oc + 1) * 128)
                            nc.tensor.matmul(
                                l1ps[oc][:], w1h_sb[:, wof], ph[:],
                                start=(k == 0), stop=(k == NK - 1),
                            )

                    def l1_cross(jc, p32, ph16, ic=ic, wc=wc):
                        pr8 = sm.tile([128, 2, NTB], f8, name="pr8", tag="pr8",
                                      bufs=4)
                        nc.vector.tensor_scalar(
                            pr8[:, 0, :], ph16[:], 1.0 / (SW * SP), None, OP.mult
                        )
                        nc.vector.scalar_tensor_tensor(
                            pr8[:, 1, :], p32[:], SP, ph16[:],
                            OP.mult, OP.subtract,
                        )
                        for oc in range(NOC):
                            wof = slice((jc * NOC + oc) * 128,
                                        (jc * NOC + oc + 1) * 128)
                            nc.tensor.matmul(
                                l1ps[oc][:], wc[:, :, wof], pr8[:, :, :],
                                start=False, stop=False, perf_mode=DR,
                            )

                    build_chunk(xt, ic, NCT1, l1_main, l1_cross, small_eng="g")

                # drain: gelu(h) = h*(0.5+0.5*erf(h/sqrt2)), h = psum/512
                for oc in range(NOC):
                    er = sm.tile([128, NTB], f32, name="er", tag="er", bufs=2)
                    nc.scalar.activation(
                        er[:], l1ps[oc][:], A.Erf,
                        scale=0.7071067811865476 / (SW * SP),
                    )
                    tg = sm.tile([128, NTB], f32, name="tg", tag="tg", bufs=2)
                    nc.vector.tensor_scalar(
                        tg[:], er[:], 0.5 / (SW * SP), 0.5 / (SW * SP),
                        OP.mult, OP.add,
                    )
                    y16 = sm.tile([128, NTB], f16, name=f"y{oc}", tag=f"y{oc}",
                                  bufs=2)
                    nc.vector.tensor_tensor(y16[:], l1ps[oc][:], tg[:], OP.mult)
                    y16s.append(y16)

                # ---------------- Layer 2 (plane-stationary) ----------------
                for ic in range(NIC):
                    def l2_main(j, ph, ic=ic):
                        k2 = ic * NTYPES + j
                        for p in range(NPC):
                            # start=True clears has_written BANK-wide, so only
                            # the very first matmul of the bank may carry it;
                            # later pieces' first writes land on hw=0 and
                            # overwrite (= write) correctly.
                            nc.tensor.matmul(
                                l2ps[0:128, p * 4 : p * 4 + 4],
                                ph[:, p * 128 : (p + 1) * 128],
                                w2m_sb[:, k2, :],
                                start=(k2 == 0 and p == 0),
                                stop=(k2 == NK - 1),
                            )

                    def l2_cross(jc, p32, ph16, ic=ic):
                        pl = sm.tile([128, NTB], f16, name="pl", tag="pl16",
                                     bufs=3)
                        nc.vector.scalar_tensor_tensor(
                            pl[:], p32[:], SP, ph16[:], OP.mult, OP.subtract
                        )
                        for p in range(NPC):
                            nc.tensor.matmul(
                                l2ps[0:128, p * 4 : p * 4 + 2],
                                pl[:, p * 128 : (p + 1) * 128],
                                w2c_sb[:, ic * NCT2 + jc, :],
                                start=False,
                                stop=False,
                            )

                    build_chunk(y16s[ic], ic, NCT2, l2_main, l2_cross,
                                small_eng="g")

                # combine hi + lo/SLO cols -> [128, NPC*2] and store
                obl = sm.tile([128, NPC, L], f32, name="obl", tag="obl", bufs=2)
                nc.vector.tensor_scalar(
                    obl[:, :, :],
                    l2ps[0:128, :].rearrange("p (n f) -> p n f", f=4)[:, :, 2:4],
                    1.0 / SLO, None, OP.mult,
                )
                ob = sm.tile([128, NPC, L], f32, name="ob", tag="ob", bufs=2)
                nc.vector.tensor_tensor(
                    ob[:, :, :], obl[:, :, :],
                    l2ps[0:128, :].rearrange("p (n f) -> p n f", f=4)[:, :, 0:2],
                    OP.add,
                )
                nc.sync.dma_start(
                    out_d[nt * NTB : (nt + 1) * NTB, :].rearrange(
                        "(n q) f -> q n f", q=128
                    ),
                    ob[:, :, :],
                )

    nc.compile()
    return nc


def _get_program():
    global _PROGRAM
    if _PROGRAM is None:
        _PROGRAM = _build_program()
    return _PROGRAM


def run(hidden, base_w1, spline_w1, scaler1, base_w2, spline_w2, scaler2, **kw):
    """Builds inputs, runs the SPMD kernel on 8 cores. Returns (output, results)."""
    from concourse.bass_utils import run_bass_kernel_spmd

    nc = _get_program()
    x = np.ascontiguousarray(np.asarray(hidden, dtype=np.float32).reshape(NTOK, H))
    w1h, w1c, w2m, w2c = _pack_weights(
        np.asarray(base_w1), np.asarray(spline_w1), np.asarray(scaler1),
        np.asarray(base_w2), np.asarray(spline_w2), np.asarray(scaler2),
    )
    in_maps = []
    for c in range(NCORES):
        xT = np.ascontiguousarray(x[c * TPC : (c + 1) * TPC].T)
        in_maps.append(
            {"xT": xT, "w1h": w1h, "w1c": w1c, "w2m": w2m, "w2c": w2c}
        )
    res = run_bass_kernel_spmd(nc, in_maps, list(range(NCORES)), **kw)
    outs = [r["outT"] for r in res.results]  # each (2048, 2)
    out = np.concatenate(outs, axis=0).reshape(B, S, L).astype(np.float32)
    return out, res


def kernel(**inputs):
    out, _ = run(**inputs)
    return out


# revision 12
# speedup vs baseline: 1.0392x; 1.0030x over previous
"""Trainium2 Bass kernel for nn_Classifier_87256555586283 (KAN 2-layer MLP). v4

Math: each kan_linear(x) = silu(x) @ base_w.T + einsum('nig,oig->no', B(x), spline_w*scaler)
where B(x) are 8 cubic B-spline bases on a uniform grid (knots 0.4 apart).

Reformulation: with u = clip(2.5*x + 5.5, 0, 11), the 8 bases are an exact linear
combination of 11 one-sided cubes phi_s(u) = relu(u - s)^3, s=0..10.  The 8->11
transform is folded into the weights host-side, so each layer becomes 12
elementwise planes (silu + 11 cubes) feeding a dense contraction with K = 12*768.

Precision: fp16 planes/weights (pre-scaled SP=32 / SW=16) + fp8e4 DoubleRow
cross matmuls carrying both residual terms (Wl8@Ph8 + Wh8@Pl8) for the
large-magnitude planes.  v4 ablation (host-emulated): L1 cross for s=0..6 only,
L2 cross s=0..4; end-to-end rel err 9.6e-3 (budget 2e-2).

Layer 2 (768->2) v4: plane-stationary matmuls.  LDWEIGHTS loads a 128-token
slice of the fp16 plane as the stationary operand; the moving operand is the
tiny [128, 4] weight slice [w2hi(2) | w2lo*256(2)], accumulating PSUM [tok, 4]
per token-piece.  Cross planes add [128, 2] w2hi moving against the
plane-residual stationary.  This replaces the M=2 512-cycle matmuls (280us of
cold PE time) with ~60-cycle N=4 matmuls whose LDWEIGHTS overlap main work.
w2lo is scaled 2^8 into its own psum columns (fp16 subnormal avoidance); the
final combine is one stt: out = hi + 2^-8 * lo.

Engine balance (per-chunk, measured costs: DVE ts 334 / stt 600, ACT 612-700,
GP tt 1230): d32+pr80+pr81 on DVE, e32 (Square) + ph (Copy*SP) on ACT,
p32 products on GpSimd.  One ACT table set (sigmoid_and_others), zero switches.

Sharding: data-parallel over the 16384 tokens across 8 cores, weights
replicated, no collectives.  x^T staged host-side (feature-major).
"""

import math

import numpy as np

# problem constants (hardcoded per contract)
B, S, H, L = 32, 512, 768, 2
NTOK = B * S            # 16384
NCORES = 8
TPC = NTOK // NCORES    # 2048 tokens per core
NTB = 512               # token block (PSUM bank = 512 fp32)
NNT = TPC // NTB        # 4
NPC = NTB // 128        # 4 token pieces per block (L2 stationary)
NS = 11                 # relu-cube feature planes
NIC = H // 128          # 6
NOC = H // 128          # 6
NTYPES = 12             # [base, s0..s10]
NK = NIC * NTYPES       # 72 K-chunks of 128
NCT1 = 6                # L1 cross planes: s = 0..5
NCT2 = 5                # L2 cross planes: s = 0..4
SW = 16.0               # weight scale 2^4
SP = 32.0               # plane scale 2^5
SLO = 256.0             # L2 lo-weight boost 2^8
RT32 = float(np.sqrt(np.float32(32.0)))

_PROGRAM = None


def _basis_transform():
    C = np.zeros((8, 12), np.float64)
    for g in range(8):
        for r in range(5):
            C[g, g + r] = ((-1) ** r) * math.comb(4, r) / 6.0
    return C[:, :11]


def _pack_weights(base_w1, spline_w1, scaler1, base_w2, spline_w2, scaler2):
    import ml_dtypes

    f8t = ml_dtypes.float8_e4m3
    C = _basis_transform()

    def full_w(bw, W, sc):
        Wp = np.einsum(
            "oig,gs->ois",
            W.astype(np.float64) * sc[..., None].astype(np.float64), C,
        ).astype(np.float32)
        return np.concatenate(
            [bw.astype(np.float32)[:, :, None], Wp], axis=2
        )  # (O, H, 12) order [base, s0..s10]

    W1 = full_w(base_w1, spline_w1, scaler1)  # (768, 768, 12)
    W2 = full_w(base_w2, spline_w2, scaler2)  # (2, 768, 12)

    W1s = (W1 * np.float32(SW)).astype(np.float32)
    W1h = W1s.astype(np.float16)
    W1l8 = ((W1s - W1h.astype(np.float32)) * np.float32(SP * SW)).astype(f8t)
    W1h8 = W1s.astype(f8t)

    # main fp16 pack: [128, NK*NOC*128], col ((ic*12+j)*NOC+oc)*128
    w1h = np.empty((128, NK * NOC * 128), np.float16)
    for ic in range(NIC):
        isl = slice(ic * 128, (ic + 1) * 128)
        for j in range(NTYPES):
            k = ic * NTYPES + j
            for oc in range(NOC):
                osl = slice(oc * 128, (oc + 1) * 128)
                w1h[:, (k * NOC + oc) * 128 : (k * NOC + oc + 1) * 128] = (
                    W1h[osl, isl, j].T
                )

    # cross fp8 pack: [128, NIC*2, NCT1*NOC*128]; dim1 = (ic, slot);
    # slot 0 = Wl8 (pairs Ph8), slot 1 = Wh8 (pairs Pl8); j = 1..NCT1 (s=0..5)
    w1c = np.zeros((128, NIC * 2, NCT1 * NOC * 128), f8t)
    for ic in range(NIC):
        isl = slice(ic * 128, (ic + 1) * 128)
        for jc in range(NCT1):
            for oc in range(NOC):
                osl = slice(oc * 128, (oc + 1) * 128)
                csl = slice((jc * NOC + oc) * 128, (jc * NOC + oc + 1) * 128)
                w1c[:, ic * 2 + 0, csl] = W1l8[osl, isl, 1 + jc].T
                w1c[:, ic * 2 + 1, csl] = W1h8[osl, isl, 1 + jc].T

    # L2 moving packs (plane-stationary matmuls):
    # w2m [128, NK, 4] = [w2hi(2) | w2lo*SLO(2)], scale 1/SP so psum = h2.
    W2n = W2.astype(np.float64) / SP          # (2, 768, 12)
    W2hi = W2n.astype(np.float16)
    W2lo = ((W2n - W2hi.astype(np.float64)) * SLO).astype(np.float16)
    w2m = np.zeros((128, NK, 4), np.float16)
    for ic in range(NIC):
        isl = slice(ic * 128, (ic + 1) * 128)
        for j in range(NTYPES):
            k2 = ic * NTYPES + j
            w2m[:, k2, 0:2] = W2hi[:, isl, j].T
            w2m[:, k2, 2:4] = W2lo[:, isl, j].T
    # cross (plane-residual) moving pack: w2c [128, NIC*NCT2, 2] = w2hi of s-plane
    w2c = np.zeros((128, NIC * NCT2, 2), np.float16)
    for ic in range(NIC):
        isl = slice(ic * 128, (ic + 1) * 128)
        for jc in range(NCT2):
            w2c[:, ic * NCT2 + jc, :] = W2hi[:, isl, 1 + jc].T
    return (
        np.ascontiguousarray(w1h),
        np.ascontiguousarray(w1c),
        np.ascontiguousarray(w2m),
        np.ascontiguousarray(w2c),
    )


def _build_program():
    import concourse.bass as bass  # noqa: F401
    import concourse.tile as tile
    from concourse import bacc, mybir

    f32 = mybir.dt.float32
    f16 = mybir.dt.float16
    f8 = mybir.dt.float8e4
    A = mybir.ActivationFunctionType
    OP = mybir.AluOpType
    DR = mybir.MatmulPerfMode.DoubleRow

    nc = bacc.Bacc(None, target_bir_lowering=False, debug=False)
    # activation() float biases need registered const APs ([128,1] SBUF)
    bias_vals = {0.0, 0.5 / (SW * SP)}
    for s in range(1, NS):
        bias_vals.add(-float(s))
        bias_vals.add(-RT32 * s)
    for val in sorted(bias_vals):
        t = nc.alloc_sbuf_tensor(f"constb-{val}", [128, 1], f32)
        nc.gpsimd.memset(t.ap(), val)
        nc.const_aps.aps[(f32, val)] = t.ap()
    nc.all_engine_barrier()

    xT_d = nc.dram_tensor("xT", [H, TPC], f32, kind="ExternalInput")
    w1h_d = nc.dram_tensor("w1h", [128, NK * NOC * 128], f16, kind="ExternalInput")
    w1c_d = nc.dram_tensor(
        "w1c", [128, NIC * 2, NCT1 * NOC * 128], f8, kind="ExternalInput"
    )
    w2m_d = nc.dram_tensor("w2m", [128, NK, 4], f16, kind="ExternalInput")
    w2c_d = nc.dram_tensor("w2c", [128, NIC * NCT2, 2], f16, kind="ExternalInput")
    out_d = nc.dram_tensor("outT", [TPC, L], f32, kind="ExternalOutput")

    with tile.TileContext(nc) as tc:
        from contextlib import ExitStack

        with ExitStack() as ctx:
            wpool = ctx.enter_context(tc.tile_pool(name="wp", bufs=1))
            w1h_sb = wpool.tile([128, NK * NOC * 128], f16, name="w1h_sb")
            blk = 6 * NOC * 128
            for c in range(12):
                csl = slice(c * blk, (c + 1) * blk)
                nc.sync.dma_start(w1h_sb[:, csl], w1h_d[:, csl])
            w2m_sb = wpool.tile([128, NK, 4], f16, name="w2m_sb")
            w2c_sb = wpool.tile([128, NIC * NCT2, 2], f16, name="w2c_sb")
            nc.sync.dma_start(w2m_sb[:], w2m_d[:])
            nc.sync.dma_start(w2c_sb[:], w2c_d[:])

            psum = ctx.enter_context(tc.tile_pool(name="psum", bufs=1, space="PSUM"))
            l1ps = [
                psum.tile([128, NTB], f32, name=f"l1ps{oc}", tag=f"l1ps{oc}")
                for oc in range(NOC)
            ]
            sm = ctx.enter_context(tc.tile_pool(name="sm", bufs=1))
            wcp = ctx.enter_context(tc.tile_pool(name="wcp", bufs=1))

            def build_chunk(src, ic, ncross, consume_main, consume_cross,
                            small_eng):
                """Produce plane tiles for one (input chunk, token block).
                src: [128, NTB] input (f32 x for L1, f16 y for L2).
                consume_main(j, ph), consume_cross(jc, p32, ph16).
                Cross planes s=0..ncross-1; small planes s=ncross..10."""
                t0 = sm.tile([128, NTB], f32, name="t0", tag="t0", bufs=3)
                u = sm.tile([128, NTB], f32, name="u", tag="u", bufs=3)
                sg = sm.tile([128, NTB], f32, name="sg", tag="sg", bufs=3)
                nc.vector.tensor_scalar(t0[:], src[:], 2.5, 5.5, OP.mult, OP.add)
                nc.vector.tensor_scalar(u[:], t0[:], 0.0, 11.0, OP.max, OP.min)
                nc.scalar.activation(sg[:], src[:], A.Sigmoid)
                pb = sm.tile([128, NTB], f16, name="pb", tag="p16", bufs=5)
                nc.vector.scalar_tensor_tensor(
                    pb[:], src[:], SP, sg[:], OP.mult, OP.mult
                )
                consume_main(0, pb)

                for s in range(ncross):  # cross planes
                    if s == 0:
                        d32 = u
                    else:
                        d32 = sm.tile([128, NTB], f32, name=f"d{s}", tag="d32",
                                      bufs=3)
                        if s % 2 == 1:
                            nc.scalar.activation(
                                d32[:], u[:], A.Relu, bias=-float(s), scale=1.0
                            )
                        else:
                            nc.vector.tensor_scalar(
                                d32[:], u[:], -float(s), 0.0, OP.add, OP.max
                            )
                    e32 = sm.tile([128, NTB], f32, name=f"e{s}", tag="e32", bufs=3)
                    nc.scalar.activation(
                        e32[:], u[:], A.Square, bias=-float(s), scale=1.0
                    )
                    p32 = sm.tile([128, NTB], f32, name=f"p{s}", tag="p32", bufs=3)
                    if s % 3 == 2:
                        nc.vector.tensor_tensor(p32[:], e32[:], d32[:], OP.mult)
                    else:
                        nc.gpsimd.tensor_tensor(p32[:], e32[:], d32[:], OP.mult)
                    ph = sm.tile([128, NTB], f16, name=f"ph{s}", tag="p16", bufs=5)
                    nc.scalar.mul(ph[:], p32[:], SP)
                    consume_main(1 + s, ph)
                    consume_cross(s, p32, ph)

                for s in range(ncross, NS):  # small planes: fp16-only
                    d32 = sm.tile([128, NTB], f32, name=f"d{s}", tag="d32", bufs=3)
                    e32 = sm.tile([128, NTB], f32, name=f"e{s}", tag="e32", bufs=3)
                    if s % 2 == 1:
                        nc.scalar.activation(
                            d32[:], u[:], A.Relu, bias=-float(s), scale=1.0
                        )
                    else:
                        nc.vector.tensor_scalar(
                            d32[:], u[:], -float(s), 0.0, OP.add, OP.max
                        )
                    nc.scalar.activation(
                        e32[:], u[:], A.Square, bias=-RT32 * s, scale=RT32
                    )
                    ph = sm.tile([128, NTB], f16, name=f"ph{s}", tag="p16", bufs=5)
                    if small_eng == "g":
                        nc.gpsimd.tensor_tensor(ph[:], e32[:], d32[:], OP.mult)
                    else:
                        nc.vector.tensor_tensor(ph[:], e32[:], d32[:], OP.mult)
                    consume_main(1 + s, ph)

            for nt in range(NNT):
                tsl = slice(nt * NTB, (nt + 1) * NTB)
                y16s = []

                l2ps = psum.tile([128, 16], f32, name="l2ps", tag="l2ps", bufs=2)

                # ---------------- Layer 1 ----------------
                for ic in range(NIC):
                    wc = wcp.tile([128, 2, NCT1 * NOC * 128], f8, name="wc",
                                  tag="wc", bufs=2)
                    nc.sync.dma_start(
                        wc[:, :, :], w1c_d[:, ic * 2 : ic * 2 + 2, :]
                    )
                    xt = sm.tile([128, NTB], f32, name="xt", tag="xt", bufs=3)
                    nc.sync.dma_start(xt[:], xT_d[ic * 128 : (ic + 1) * 128, tsl])

                    def l1_main(j, ph, ic=ic):
                        k = ic * NTYPES + j
                        for oc in range(NOC):
                            wof = slice((k * NOC + oc) * 128,
                                        (k * NOC + oc + 1) * 128)
                            nc.tensor.matmul(
                                l1ps[oc][:], w1h_sb[:, wof], ph[:],
                                start=(k == 0), stop=(k == NK - 1),
                            )

                    def l1_cross(jc, p32, ph16, ic=ic, wc=wc):
                        pr8 = sm.tile([128, 2, NTB], f8, name="pr8", tag="pr8",
                                      bufs=4)
                        nc.vector.tensor_scalar(
                            pr8[:, 0, :], ph16[:], 1.0 / (SW * SP), None, OP.mult
                        )
                        nc.vector.scalar_tensor_tensor(
                            pr8[:, 1, :], p32[:], SP, ph16[:],
                            OP.mult, OP.subtract,
                        )
                        for oc in range(NOC):
                            wof = slice((jc * NOC + oc) * 128,
                                        (jc * NOC + oc + 1) * 128)
                            nc.tensor.matmul(
                                l1ps[oc][:], wc[:, :, wof], pr8[:, :, :],
                                start=False, stop=False, perf_mode=DR,
                            )

                    build_chunk(xt, ic, NCT1, l1_main, l1_cross, small_eng="g")

                # drain: gelu(h) = h*(0.5+0.5*erf(h/sqrt2)), h = psum/512
                for oc in range(NOC):
                    er = sm.tile([128, NTB], f32, name="er", tag="er", bufs=2)
                    nc.scalar.activation(
                        er[:], l1ps[oc][:], A.Erf,
                        scale=0.7071067811865476 / (SW * SP),
                    )
                    tg = sm.tile([128, NTB], f32, name="tg", tag="tg", bufs=2)
                    nc.scalar.activation(
                        tg[:], er[:], A.Copy,
                        bias=0.5 / (SW * SP), scale=0.5 / (SW * SP),
                    )
                    y16 = sm.tile([128, NTB], f16, name=f"y{oc}", tag=f"y{oc}",
                                  bufs=2)
                    nc.vector.tensor_tensor(y16[:], l1ps[oc][:], tg[:], OP.mult)
                    y16s.append(y16)

                # ---------------- Layer 2 (plane-stationary) ----------------
                for ic in range(NIC):
                    def l2_main(j, ph, ic=ic):
                        k2 = ic * NTYPES + j
                        for p in range(NPC):
                            # start=True clears has_written BANK-wide, so only
                            # the very first matmul of the bank may carry it;
                            # later pieces' first writes land on hw=0 and
                            # overwrite (= write) correctly.
                            nc.tensor.matmul(
                                l2ps[0:128, p * 4 : p * 4 + 4],
                                ph[:, p * 128 : (p + 1) * 128],
                                w2m_sb[:, k2, :],
                                start=(k2 == 0 and p == 0),
                                stop=(k2 == NK - 1),
                            )

                    def l2_cross(jc, p32, ph16, ic=ic):
                        pl = sm.tile([128, NTB], f16, name="pl", tag="pl16",
                                     bufs=3)
                        nc.vector.scalar_tensor_tensor(
                            pl[:], p32[:], SP, ph16[:], OP.mult, OP.subtract
                        )
                        for p in range(NPC):
                            nc.tensor.matmul(
                                l2ps[0:128, p * 4 : p * 4 + 2],
                                pl[:, p * 128 : (p + 1) * 128],
                                w2c_sb[:, ic * NCT2 + jc, :],
                                start=False,
                                stop=False,
                            )

                    build_chunk(y16s[ic], ic, NCT2, l2_main, l2_cross,
                                small_eng="g")

                # combine hi + lo/SLO cols -> [128, NPC*2] and store
                obl = sm.tile([128, NPC, L], f32, name="obl", tag="obl", bufs=2)
                nc.vector.tensor_scalar(
                    obl[:, :, :],
                    l2ps[0:128, :].rearrange("p (n f) -> p n f", f=4)[:, :, 2:4],
                    1.0 / SLO, None, OP.mult,
                )
                ob = sm.tile([128, NPC, L], f32, name="ob", tag="ob", bufs=2)
                nc.vector.tensor_tensor(
                    ob[:, :, :], obl[:, :, :],
                    l2ps[0:128, :].rearrange("p (n f) -> p n f", f=4)[:, :, 0:2],
                    OP.add,
                )
                nc.sync.dma_start(
                    out_d[nt * NTB : (nt + 1) * NTB, :].rearrange(
                        "(n q) f -> q n f", q=128
                    ),
                    ob[:, :, :],
                )

    nc.compile()
    return nc


def _get_program():
    global _PROGRAM
    if _PROGRAM is None:
        _PROGRAM = _build_program()
    return _PROGRAM


def run(hidden, base_w1, spline_w1, scaler1, base_w2, spline_w2, scaler2, **kw):
    """Builds inputs, runs the SPMD kernel on 8 cores. Returns (output, results)."""
    from concourse.bass_utils import run_bass_kernel_spmd

    nc = _get_program()
    x = np.ascontiguousarray(np.asarray(hidden, dtype=np.float32).reshape(NTOK, H))
    w1h, w1c, w2m, w2c = _pack_weights(
        np.asarray(base_w1), np.asarray(spline_w1), np.asarray(scaler1),
        np.asarray(base_w2), np.asarray(spline_w2), np.asarray(scaler2),
    )
    in_maps = []
    for c in range(NCORES):
        xT = np.ascontiguousarray(x[c * TPC : (c + 1) * TPC].T)
        in_maps.append(
            {"xT": xT, "w1h": w1h, "w1c": w1c, "w2m": w2m, "w2c": w2c}
        )
    res = run_bass_kernel_spmd(nc, in_maps, list(range(NCORES)), **kw)
    outs = [r["outT"] for r in res.results]  # each (2048, 2)
    out = np.concatenate(outs, axis=0).reshape(B, S, L).astype(np.float32)
    return out, res


def kernel(**inputs):
    out, _ = run(**inputs)
    return out
